# revision 19
# baseline (speedup 1.0000x reference)
"""Trainium2 Bass kernel for causal self-attention with LoRA (q,v adapters).

Full-input contract: kernel(**inputs) takes the unsharded numpy inputs and
returns (a, present) exactly like the reference.

Sharding: 8 cores = 2 batches x 4 head-groups (4 heads each).  Data parallel
on B, tensor parallel on heads: each core owns 256 columns of q/k/v and the
matching 256 rows of c_proj.  LoRA adapters are folded into c_attn_w on the
host (exact: q_eff = x @ (Wq + 0.5*q_a1@q_a2)).  The c_proj partial outputs
are summed on the host.

Device dataflow per core (all matmuls bf16 with fp32 PSUM accumulation):
  xT [1024,2048] -> QT,KT [256(cols),2048] transposed layout, V [2048,256]
  natural layout with a ones-column appended per head (softmax denominator
  comes out of the P@V matmul for free).
  Scores are computed transposed, ST[k,q] = KT_h^T slice . QT_h, so that
  P^T tiles feed the P@V matmul directly (contraction over k on partitions;
  no on-chip transposes anywhere in the kernel).
  exp(s/8) runs on ScalarE with the 1/sqrt(hd) scale folded in; causal
  masking multiplies the diagonal P^T tiles by 0/1 masks generated on-device.
  Normalization by the softmax denominator happens on the [65,512] P@V
  output via reciprocal + K=1 outer-product broadcast matmul.
  Finally projT: a_part = (aT)^T @ Wproj_rows accumulated in PSUM.

Bias handling (exact for arbitrary biases):
  q,k bias: added on device per-partition during the PSUM->SBUF copies.
  v bias: softmax rows sum to 1 so attention output shifts by exactly +bv;
  folded on host into `a` as bv @ c_proj_w, and added to present-v directly.
  proj bias: host-added.
"""

import os
import numpy as np
import ml_dtypes

import concourse.bass as bass
import concourse.tile as tile
from concourse.tile_rust import add_dep_helper
import concourse.mybir as mybir
from concourse import bacc
from concourse.bass_utils import run_bass_kernel_spmd

BF16 = mybir.dt.bfloat16
F32 = mybir.dt.float32
AF = mybir.ActivationFunctionType
NPBF16 = ml_dtypes.bfloat16

B, S, D = 2, 2048, 1024
N_HEAD = 16
HD = D // N_HEAD              # 64
LORA_SCALE = 16.0 / 32.0
N_CORES = 8
HPC = 4                       # heads per core
GC = HPC * HD                 # local columns per core = 256
KCH = D // 128                # contraction chunks = 8
NTB = S // 512                # 512-wide token blocks = 4
NTB128 = S // 128             # 128-wide token blocks = 16


def _build_program(with_bias=False):
    nc = bacc.Bacc("TRN2", target_bir_lowering=False, debug=False)

    # ---- per-core DRAM I/O ----
    xt_d = nc.dram_tensor("xt", [NTB, D, 512], BF16, kind="ExternalInput")
    wq_d = nc.dram_tensor("wq", [D, GC], BF16, kind="ExternalInput")
    wk_d = nc.dram_tensor("wk", [D, GC], BF16, kind="ExternalInput")
    wv_d = nc.dram_tensor("wv", [D, GC], BF16, kind="ExternalInput")
    wp_d = nc.dram_tensor("wp", [GC, D], BF16, kind="ExternalInput")
    qb_d = nc.dram_tensor("qb", [128, 2], F32, kind="ExternalInput")
    kb_d = nc.dram_tensor("kb", [128, 2], F32, kind="ExternalInput")

    a_d = nc.dram_tensor("a_part", [2, S, 512], BF16, kind="ExternalOutput")
    k_d = nc.dram_tensor("k_out", [2, NTB, 128, 512], BF16, kind="ExternalOutput")
    v_d = nc.dram_tensor("v_out", [S, GC], BF16, kind="ExternalOutput")
    debug = bool(int(os.environ.get("BASSK_DEBUG", "0")))
    if debug:
        qt_dbg = nc.dram_tensor("qt_dbg", [2, 128, S], BF16, kind="ExternalOutput")
        kt_dbg = nc.dram_tensor("kt_dbg", [2, 128, S], BF16, kind="ExternalOutput")
        vx_dbg = nc.dram_tensor("vx_dbg", [128, HPC * NTB128 * (HD + 1)], BF16,
                                kind="ExternalOutput")
        at_dbg = nc.dram_tensor("at_dbg", [2, 128, S], BF16, kind="ExternalOutput")

    with tile.TileContext(nc) as tc:
        # ---- persistent SBUF tiles, split per block so Tile's per-tile
        # dependency tracking lets phases pipeline into each other ----
        frees = []

        def single(shape, dtype, name):
            t, f = tc.tile(shape, dtype, name=name)
            frees.append(f)
            return t

        xt_tb = [single([128, KCH * 512], BF16, f"xt_tb{t}") for t in range(NTB)]
        wq_sb = single([128, KCH * GC], BF16, "wq_sb")
        wk_sb = single([128, KCH * GC], BF16, "wk_sb")
        wv_sb = single([128, KCH * GC], BF16, "wv_sb")
        wp_sb = single([128, 2 * D], BF16, "wp_sb")
        qb_sb = single([128, 2], F32, "qb_sb")
        kb_sb = single([128, 2], F32, "kb_sb")
        qt_tb = [[single([128, 512], BF16, f"qt_tb{c}_{t}") for t in range(NTB)]
                 for c in range(2)]
        kt_tb = [[single([128, 512], BF16, f"kt_tb{c}_{t}") for t in range(NTB)]
                 for c in range(2)]
        # vx_t[t]: V rows for tokens [t*512,(t+1)*512), all 4 heads, with a
        # ones column per 128-chunk: layout [p, (h, c4, 65)]
        vx_t = [single([128, HPC * 4 * (HD + 1)], BF16, f"vx_t{t}")
                for t in range(NTB)]
        at_tb = [[single([128, 512], BF16, f"at_tb{c}_{n}") for n in range(NTB)]
                 for c in range(2)]
        masks = [single([128, 1024], BF16, f"mask{j}") for j in range(2)]

        # ---- transient pools ----
        stage = tc.alloc_tile_pool(name="stage", bufs=4)
        ptp = tc.alloc_tile_pool(name="ptp", bufs=8)
        rsp = tc.alloc_tile_pool(name="rsp", bufs=4)
        ph1 = tc.alloc_tile_pool(name="ph1", bufs=2, space="PSUM")
        scp = tc.alloc_tile_pool(name="scp", bufs=2, space="PSUM")
        otp = tc.alloc_tile_pool(name="otp", bufs=2, space="PSUM")

        # ---- constants ----
        for t in range(NTB):
            ones_slots = vx_t[t][:].rearrange(
                "p (h c w) -> p h c w", h=HPC, w=HD + 1)[:, :, :, HD]
            nc.vector.memset(ones_slots, 1.0)
        for pj in range(2):
            # halves hold mask_{2pj} and mask_{2pj+1};
            # mask_j[r, qq] = 1.0 iff qq >= j*128 + r else 0.0
            nc.gpsimd.memset(masks[pj][:], 1.0)
            for half in range(2):
                j = 2 * pj + half
                nc.gpsimd.affine_select(
                    out=masks[pj][:, half * 512:(half + 1) * 512],
                    in_=masks[pj][:, half * 512:(half + 1) * 512],
                    compare_op=mybir.AluOpType.is_ge,
                    fill=0.0, base=-128 * j,
                    pattern=[[1, 512]], channel_multiplier=-1,
                )

        # ---- input DMAs, explicitly chained so transfers complete in
        # consumption order (parallel queues would otherwise share HBM
        # bandwidth and all finish together, stalling the first matmuls) ----
        chain = []
        for w_sb, w_d in ((wq_sb, wq_d), (wk_sb, wk_d)):
            chain.append(nc.sync.dma_start(
                w_sb[:].rearrange("p (k n) -> p k n", n=GC),
                w_d.ap().rearrange("(k p) n -> p k n", p=128)))
        chain.append(nc.sync.dma_start(
            xt_tb[0][:].rearrange("p (k w) -> p k w", w=512),
            xt_d.ap()[0].rearrange("(k p) w -> p k w", p=128)))
        chain.append(nc.scalar.dma_start(
            wv_sb[:].rearrange("p (k n) -> p k n", n=GC),
            wv_d.ap().rearrange("(k p) n -> p k n", p=128)))
        for t in range(1, NTB):
            chain.append(nc.sync.dma_start(
                xt_tb[t][:].rearrange("p (k w) -> p k w", w=512),
                xt_d.ap()[t].rearrange("(k p) w -> p k w", p=128)))
        chain.append(nc.gpsimd.dma_start(
            wp_sb[:].rearrange("p (k n) -> p k n", n=D),
            wp_d.ap().rearrange("(k p) n -> p k n", p=128)))
        for prev, nxt in zip(chain, chain[1:]):
            add_dep_helper(nxt.ins, prev.ins, sync=True,
                           reason="serialize input DMA stream")
        if with_bias:
            nc.sync.dma_start(qb_sb[:], qb_d.ap())
            nc.sync.dma_start(kb_sb[:], kb_d.ap())

        # ---- phase 1: QT, KT, V interleaved per 512-token block ----
        for t in range(NTB):
            for c in range(2):
                psq = ph1.tile([128, 512], F32, tag="ph1")
                for k in range(KCH):
                    nc.tensor.matmul(
                        psq[:],
                        wq_sb[:, k * GC + c * 128: k * GC + (c + 1) * 128],
                        xt_tb[t][:, k * 512:(k + 1) * 512],
                        start=(k == 0), stop=(k == KCH - 1))
                if with_bias:
                    nc.scalar.activation(qt_tb[c][t][:], psq[:], AF.Identity,
                                         bias=qb_sb[:, c: c + 1])
                else:
                    nc.vector.tensor_copy(qt_tb[c][t][:], psq[:])

                psk = ph1.tile([128, 512], F32, tag="ph1")
                for k in range(KCH):
                    nc.tensor.matmul(
                        psk[:],
                        wk_sb[:, k * GC + c * 128: k * GC + (c + 1) * 128],
                        xt_tb[t][:, k * 512:(k + 1) * 512],
                        start=(k == 0), stop=(k == KCH - 1))
                if with_bias:
                    nc.scalar.activation(kt_tb[c][t][:], psk[:], AF.Identity,
                                         bias=kb_sb[:, c: c + 1])
                else:
                    nc.vector.tensor_copy(kt_tb[c][t][:], psk[:])
                nc.sync.dma_start(k_d.ap()[c, t], kt_tb[c][t][:])

            for q4 in range(4):
                tt = 4 * t + q4
                psv = ph1.tile([128, GC], F32, tag="ph1")
                for k in range(KCH):
                    nc.tensor.matmul(
                        psv[:],
                        xt_tb[t][:, k * 512 + q4 * 128: k * 512 + (q4 + 1) * 128],
                        wv_sb[:, k * GC:(k + 1) * GC],
                        start=(k == 0), stop=(k == KCH - 1))
                vx_dst = vx_t[t][:].rearrange(
                    "p (h c w) -> p h c w", h=HPC, w=HD + 1)[:, :, q4, 0:HD]
                nc.vector.tensor_copy(
                    vx_dst, psv[:].rearrange("p (h w) -> p h w", w=HD))
                vst = stage.tile([128, GC], BF16, tag="stage")
                nc.vector.tensor_copy(vst[:], psv[:])
                nc.sync.dma_start(v_d.ap()[tt * 128:(tt + 1) * 128, :], vst[:])

        # ---- phase 2: causal attention, n-outer h-inner so each head's
        # normalize chain overlaps the other heads' matmuls ----
        for n in range(NTB):
            for h in range(HPC):
                c, poff = h // 2, (h % 2) * 64
                nkb = 4 * n + 4
                ot = otp.tile([HD + 1, 512], F32, tag="otp")
                for mp in range(nkb // 2):
                    sc = scp.tile([128, 1024], F32, tag="scp")
                    for half in range(2):
                        m = 2 * mp + half
                        nc.tensor.matmul(
                            sc[:, half * 512:(half + 1) * 512],
                            kt_tb[c][m // 4][poff:poff + 64,
                                             (m % 4) * 128:(m % 4 + 1) * 128],
                            qt_tb[c][n][poff:poff + 64, :],
                            start=True, stop=True)
                    pt = ptp.tile([128, 1024], BF16, tag="ptp")
                    nc.scalar.activation(pt[:], sc[:], AF.Exp, scale=0.125)
                    pjidx = mp - 2 * n
                    if pjidx >= 0:
                        nc.vector.tensor_mul(pt[:], pt[:], masks[pjidx][:])
                    for half in range(2):
                        m = 2 * mp + half
                        nc.tensor.matmul(
                            ot[:],
                            vx_t[m // 4][:, h * 4 * (HD + 1) + (m % 4) * (HD + 1):
                                         h * 4 * (HD + 1) + (m % 4 + 1) * (HD + 1)],
                            pt[:, half * 512:(half + 1) * 512],
                            start=(m == 0), stop=(m == nkb - 1))
                # normalize: at[poff:poff+64] = ot[0:64] * (1/ot[64])
                den = rsp.tile([1, 512], F32, tag="den")
                nc.vector.tensor_copy(den[:], ot[HD:HD + 1, :])
                rsf = rsp.tile([1, 512], F32, tag="rsf")
                nc.vector.reciprocal_approx_fast(rsf[:], den[:])
                rbc = rsp.tile([64, 512], F32, tag="rbc")
                nc.gpsimd.partition_broadcast(rbc[:], rsf[:])
                nc.vector.tensor_mul(at_tb[c][n][poff:poff + 64, :],
                                     ot[0:HD, :], rbc[:])

        if debug:
            for c in range(2):
                for t in range(NTB):
                    nc.sync.dma_start(
                        qt_dbg.ap()[c, :, t * 512:(t + 1) * 512], qt_tb[c][t][:])
                    nc.sync.dma_start(
                        kt_dbg.ap()[c, :, t * 512:(t + 1) * 512], kt_tb[c][t][:])
                    nc.sync.dma_start(
                        at_dbg.ap()[c, :, t * 512:(t + 1) * 512], at_tb[c][t][:])
            for t in range(NTB):
                nc.sync.dma_start(
                    vx_dbg.ap()[:, t * HPC * 4 * (HD + 1):
                                (t + 1) * HPC * 4 * (HD + 1)], vx_t[t][:])

        # ---- phase 3: projection partial: a_part = aT^T @ Wproj ----
        for t in range(NTB128):
            n, off = t // 4, (t % 4) * 128
            for ob in range(2):
                pj = ph1.tile([128, 512], F32, tag="ph1")
                for cc in range(2):
                    nc.tensor.matmul(
                        pj[:],
                        at_tb[cc][n][:, off:off + 128],
                        wp_sb[:, cc * D + ob * 512: cc * D + (ob + 1) * 512],
                        start=(cc == 0), stop=(cc == 1))
                pst = stage.tile([128, 512], BF16, tag="stage")
                nc.vector.tensor_copy(pst[:], pj[:])
                nc.sync.dma_start(
                    a_d.ap()[ob, t * 128:(t + 1) * 128, :], pst[:])

        for p in (otp, scp, ph1, rsp, ptp, stage):
            p.release()
        for f in reversed(frees):
            f()

    nc.compile()
    return nc


_NC_CACHE = {}


def _get_program(with_bias=False):
    key = f"nc{int(with_bias)}"
    if key not in _NC_CACHE:
        _NC_CACHE[key] = _build_program(with_bias)
    return _NC_CACHE[key]


def kernel(x, c_attn_w, c_attn_b, c_proj_w, c_proj_b, q_a1, q_a2, v_a1, v_a2):
    x = np.asarray(x, np.float32)
    c_attn_w = np.asarray(c_attn_w, np.float32)
    c_attn_b = np.asarray(c_attn_b, np.float32)
    c_proj_w = np.asarray(c_proj_w, np.float32)
    c_proj_b = np.asarray(c_proj_b, np.float32)
    q_a1 = np.asarray(q_a1, np.float32)
    q_a2 = np.asarray(q_a2, np.float32)
    v_a1 = np.asarray(v_a1, np.float32)
    v_a2 = np.asarray(v_a2, np.float32)

    # Host prep: fold LoRA into the attention weights (exact).
    wq_eff = c_attn_w[:, :D] + LORA_SCALE * (q_a1 @ q_a2)
    wk_full = c_attn_w[:, D:2 * D]
    wv_eff = c_attn_w[:, 2 * D:] + LORA_SCALE * (v_a1 @ v_a2)
    qb_full, kb_full, vb_full = (c_attn_b[:D], c_attn_b[D:2 * D],
                                 c_attn_b[2 * D:])

    in_maps = []
    for core in range(N_CORES):
        b, g = divmod(core, 4)
        cs = slice(g * GC, (g + 1) * GC)
        in_maps.append({
            "xt": np.ascontiguousarray(
                x[b].T.astype(NPBF16).reshape(D, NTB, 512).transpose(1, 0, 2)),
            "wq": np.ascontiguousarray(wq_eff[:, cs].astype(NPBF16)),
            "wk": np.ascontiguousarray(wk_full[:, cs].astype(NPBF16)),
            "wv": np.ascontiguousarray(wv_eff[:, cs].astype(NPBF16)),
            "wp": np.ascontiguousarray(c_proj_w[cs, :].astype(NPBF16)),
            "qb": np.ascontiguousarray(
                qb_full[cs].reshape(2, 128).T.astype(np.float32)),
            "kb": np.ascontiguousarray(
                kb_full[cs].reshape(2, 128).T.astype(np.float32)),
        })

    with_bias = bool(np.any(c_attn_b[:2 * D]))
    nc = _get_program(with_bias)
    trace = bool(int(os.environ.get("BASSK_TRACE", "0")))
    res = run_bass_kernel_spmd(nc, in_maps, list(range(N_CORES)), trace=trace)
    if trace:
        _NC_CACHE["exec_time_ns"] = res.exec_time_ns
        _NC_CACHE["profile_json"] = res.profile_json
        _NC_CACHE["trace"] = res.instructions_and_trace

    # Host gather.
    a = np.zeros((B, S, D), np.float32)
    kh = np.empty((B, N_HEAD, S, HD), np.float32)
    vh = np.empty((B, N_HEAD, S, HD), np.float32)
    for core in range(N_CORES):
        b, g = divmod(core, 4)
        r = res.results[core]
        ap_bl = r["a_part"].astype(np.float32)
        a[b, :, 0:512] += ap_bl[0]
        a[b, :, 512:1024] += ap_bl[1]
        for hl in range(HPC):
            h = g * HPC + hl
            c, half = hl // 2, hl % 2
            kh[b, h] = r["k_out"][c][:, half * 64:(half + 1) * 64, :].transpose(
                0, 2, 1).reshape(S, HD).astype(np.float32)
            vh[b, h] = r["v_out"][:, hl * HD:(hl + 1) * HD].astype(np.float32)

    # Exact host-side bias folds: softmax rows sum to 1, so the v bias
    # passes through attention additively -> contributes vb @ c_proj_w.
    a += (c_proj_b + vb_full @ c_proj_w).astype(np.float32)
    vh += vb_full.reshape(N_HEAD, 1, HD)[None]
    present = np.stack((kh, vh)).astype(np.float32)
    return a, present


# revision 22
# speedup vs baseline: 1.0081x; 1.0081x over previous
"""Trainium2 Bass kernel for causal self-attention with LoRA (q,v adapters).

Full-input contract: kernel(**inputs) takes the unsharded numpy inputs and
returns (a, present) exactly like the reference.

Sharding: 8 cores = 2 batches x 4 head-groups (4 heads each).  Data parallel
on B, tensor parallel on heads: each core owns 256 columns of q/k/v and the
matching 256 rows of c_proj.  LoRA adapters are folded into c_attn_w on the
host (exact: q_eff = x @ (Wq + 0.5*q_a1@q_a2)).  The c_proj partial outputs
are summed on the host.

Device dataflow per core (all matmuls bf16 with fp32 PSUM accumulation):
  xT [1024,2048] -> QT,KT [256(cols),2048] transposed layout, V [2048,256]
  natural layout with a ones-column appended per head (softmax denominator
  comes out of the P@V matmul for free).
  Scores are computed transposed, ST[k,q] = KT_h^T slice . QT_h, so that
  P^T tiles feed the P@V matmul directly (contraction over k on partitions;
  no on-chip transposes anywhere in the kernel).
  exp(s/8) runs on ScalarE with the 1/sqrt(hd) scale folded in; causal
  masking multiplies the diagonal P^T tiles by 0/1 masks generated on-device.
  Normalization by the softmax denominator happens on the [65,512] P@V
  output via reciprocal + K=1 outer-product broadcast matmul.
  Finally projT: a_part = (aT)^T @ Wproj_rows accumulated in PSUM.

Bias handling (exact for arbitrary biases):
  q,k bias: added on device per-partition during the PSUM->SBUF copies.
  v bias: softmax rows sum to 1 so attention output shifts by exactly +bv;
  folded on host into `a` as bv @ c_proj_w, and added to present-v directly.
  proj bias: host-added.
"""

import os
import numpy as np
import ml_dtypes

import concourse.bass as bass
import concourse.tile as tile
from concourse.tile_rust import add_dep_helper
import concourse.mybir as mybir
from concourse import bacc
from concourse.bass_utils import run_bass_kernel_spmd

BF16 = mybir.dt.bfloat16
F32 = mybir.dt.float32
AF = mybir.ActivationFunctionType
NPBF16 = ml_dtypes.bfloat16

B, S, D = 2, 2048, 1024
N_HEAD = 16
HD = D // N_HEAD              # 64
LORA_SCALE = 16.0 / 32.0
N_CORES = 8
HPC = 4                       # heads per core
GC = HPC * HD                 # local columns per core = 256
KCH = D // 128                # contraction chunks = 8
NTB = S // 512                # 512-wide token blocks = 4
NTB128 = S // 128             # 128-wide token blocks = 16


def _build_program(with_bias=False):
    nc = bacc.Bacc("TRN2", target_bir_lowering=False, debug=False)

    # ---- per-core DRAM I/O ----
    xt_d = nc.dram_tensor("xt", [NTB, D, 512], BF16, kind="ExternalInput")
    wq_d = nc.dram_tensor("wq", [D, GC], BF16, kind="ExternalInput")
    wk_d = nc.dram_tensor("wk", [D, GC], BF16, kind="ExternalInput")
    wv_d = nc.dram_tensor("wv", [D, GC], BF16, kind="ExternalInput")
    wp_d = nc.dram_tensor("wp", [GC, D], BF16, kind="ExternalInput")
    qb_d = nc.dram_tensor("qb", [128, 2], F32, kind="ExternalInput")
    kb_d = nc.dram_tensor("kb", [128, 2], F32, kind="ExternalInput")

    a_d = nc.dram_tensor("a_part", [2, S, 512], BF16, kind="ExternalOutput")
    k_d = nc.dram_tensor("k_out", [2, NTB, 128, 512], BF16, kind="ExternalOutput")
    v_d = nc.dram_tensor("v_out", [S, GC], BF16, kind="ExternalOutput")
    debug = bool(int(os.environ.get("BASSK_DEBUG", "0")))
    if debug:
        qt_dbg = nc.dram_tensor("qt_dbg", [2, 128, S], BF16, kind="ExternalOutput")
        kt_dbg = nc.dram_tensor("kt_dbg", [2, 128, S], BF16, kind="ExternalOutput")
        vx_dbg = nc.dram_tensor("vx_dbg", [128, HPC * NTB128 * (HD + 1)], BF16,
                                kind="ExternalOutput")
        at_dbg = nc.dram_tensor("at_dbg", [2, 128, S], BF16, kind="ExternalOutput")

    with tile.TileContext(nc) as tc:
        # ---- persistent SBUF tiles, split per block so Tile's per-tile
        # dependency tracking lets phases pipeline into each other ----
        frees = []

        def single(shape, dtype, name):
            t, f = tc.tile(shape, dtype, name=name)
            frees.append(f)
            return t

        xt_tb0 = single([128, KCH * 512], BF16, "xt_tb0")
        xt_rest = single([128, (NTB - 1) * KCH * 512], BF16, "xt_rest")

        def xt_slice(t, k, off, width):
            if t == 0:
                return xt_tb0[:, k * 512 + off: k * 512 + off + width]
            base = (t - 1) * KCH * 512 + k * 512 + off
            return xt_rest[:, base: base + width]
        wq_sb = single([128, KCH * GC], BF16, "wq_sb")
        wk_sb = single([128, KCH * GC], BF16, "wk_sb")
        wv_sb = single([128, KCH * GC], BF16, "wv_sb")
        wp_sb = single([128, 2 * D], BF16, "wp_sb")
        qb_sb = single([128, 2], F32, "qb_sb")
        kb_sb = single([128, 2], F32, "kb_sb")
        qt_tb = [[single([128, 512], BF16, f"qt_tb{c}_{t}") for t in range(NTB)]
                 for c in range(2)]
        kt_tb = [[single([128, 512], BF16, f"kt_tb{c}_{t}") for t in range(NTB)]
                 for c in range(2)]
        # vx_t[t]: V rows for tokens [t*512,(t+1)*512), all 4 heads, with a
        # ones column per 128-chunk: layout [p, (h, c4, 65)]
        vx_t = [single([128, HPC * 4 * (HD + 1)], BF16, f"vx_t{t}")
                for t in range(NTB)]
        at_tb = [[single([128, 512], BF16, f"at_tb{c}_{n}") for n in range(NTB)]
                 for c in range(2)]
        masks = [single([128, 1024], BF16, f"mask{j}") for j in range(2)]

        # ---- transient pools ----
        stage = tc.alloc_tile_pool(name="stage", bufs=4)
        ptp = tc.alloc_tile_pool(name="ptp", bufs=8)
        rsp = tc.alloc_tile_pool(name="rsp", bufs=4)
        ph1 = tc.alloc_tile_pool(name="ph1", bufs=2, space="PSUM")
        scp = tc.alloc_tile_pool(name="scp", bufs=2, space="PSUM")
        otp = tc.alloc_tile_pool(name="otp", bufs=2, space="PSUM")

        # ---- constants ----
        for t in range(NTB):
            ones_slots = vx_t[t][:].rearrange(
                "p (h c w) -> p h c w", h=HPC, w=HD + 1)[:, :, :, HD]
            nc.vector.memset(ones_slots, 1.0)
        for pj in range(2):
            # halves hold mask_{2pj} and mask_{2pj+1};
            # mask_j[r, qq] = 1.0 iff qq >= j*128 + r else 0.0
            nc.gpsimd.memset(masks[pj][:], 1.0)
            for half in range(2):
                j = 2 * pj + half
                nc.gpsimd.affine_select(
                    out=masks[pj][:, half * 512:(half + 1) * 512],
                    in_=masks[pj][:, half * 512:(half + 1) * 512],
                    compare_op=mybir.AluOpType.is_ge,
                    fill=0.0, base=-128 * j,
                    pattern=[[1, 512]], channel_multiplier=-1,
                )

        # ---- input DMAs: xt block 0 gets its own transfer so it lands
        # first; blocks 1-3 stream as one big transfer behind it ----
        for w_sb, w_d in ((wq_sb, wq_d), (wk_sb, wk_d)):
            nc.sync.dma_start(
                w_sb[:].rearrange("p (k n) -> p k n", n=GC),
                w_d.ap().rearrange("(k p) n -> p k n", p=128))
        nc.sync.dma_start(
            xt_tb0[:].rearrange("p (k w) -> p k w", w=512),
            xt_d.ap()[0].rearrange("(k p) w -> p k w", p=128))
        nc.sync.dma_start(
            xt_rest[:].rearrange("p (t k w) -> p t k w", t=NTB - 1, w=512),
            xt_d.ap()[1:NTB].rearrange("t (k p) w -> p t k w", p=128))
        nc.scalar.dma_start(
            wv_sb[:].rearrange("p (k n) -> p k n", n=GC),
            wv_d.ap().rearrange("(k p) n -> p k n", p=128))
        nc.gpsimd.dma_start(
            wp_sb[:].rearrange("p (k n) -> p k n", n=D),
            wp_d.ap().rearrange("(k p) n -> p k n", p=128))
        if with_bias:
            nc.sync.dma_start(qb_sb[:], qb_d.ap())
            nc.sync.dma_start(kb_sb[:], kb_d.ap())

        # ---- phase 1: QT, KT, V interleaved per 512-token block ----
        for t in range(NTB):
            for c in range(2):
                psq = ph1.tile([128, 512], F32, tag="ph1")
                for k in range(KCH):
                    nc.tensor.matmul(
                        psq[:],
                        wq_sb[:, k * GC + c * 128: k * GC + (c + 1) * 128],
                        xt_slice(t, k, 0, 512),
                        start=(k == 0), stop=(k == KCH - 1))
                if with_bias:
                    nc.scalar.activation(qt_tb[c][t][:], psq[:], AF.Identity,
                                         bias=qb_sb[:, c: c + 1])
                else:
                    nc.vector.tensor_copy(qt_tb[c][t][:], psq[:])

                psk = ph1.tile([128, 512], F32, tag="ph1")
                for k in range(KCH):
                    nc.tensor.matmul(
                        psk[:],
                        wk_sb[:, k * GC + c * 128: k * GC + (c + 1) * 128],
                        xt_slice(t, k, 0, 512),
                        start=(k == 0), stop=(k == KCH - 1))
                if with_bias:
                    nc.scalar.activation(kt_tb[c][t][:], psk[:], AF.Identity,
                                         bias=kb_sb[:, c: c + 1])
                else:
                    nc.vector.tensor_copy(kt_tb[c][t][:], psk[:])
                nc.sync.dma_start(k_d.ap()[c, t], kt_tb[c][t][:])

            for q4 in range(4):
                tt = 4 * t + q4
                psv = ph1.tile([128, GC], F32, tag="ph1")
                for k in range(KCH):
                    nc.tensor.matmul(
                        psv[:],
                        xt_slice(t, k, q4 * 128, 128),
                        wv_sb[:, k * GC:(k + 1) * GC],
                        start=(k == 0), stop=(k == KCH - 1))
                vx_dst = vx_t[t][:].rearrange(
                    "p (h c w) -> p h c w", h=HPC, w=HD + 1)[:, :, q4, 0:HD]
                nc.vector.tensor_copy(
                    vx_dst, psv[:].rearrange("p (h w) -> p h w", w=HD))
                vst = stage.tile([128, GC], BF16, tag="stage")
                nc.vector.tensor_copy(vst[:], psv[:])
                nc.sync.dma_start(v_d.ap()[tt * 128:(tt + 1) * 128, :], vst[:])

        # ---- phase 2: causal attention, n-outer h-inner so each head's
        # normalize chain overlaps the other heads' matmuls ----
        for n in range(NTB):
            for h in range(HPC):
                c, poff = h // 2, (h % 2) * 64
                nkb = 4 * n + 4
                ot = otp.tile([HD + 1, 512], F32, tag="otp")
                for mp in range(nkb // 2):
                    sc = scp.tile([128, 1024], F32, tag="scp")
                    for half in range(2):
                        m = 2 * mp + half
                        nc.tensor.matmul(
                            sc[:, half * 512:(half + 1) * 512],
                            kt_tb[c][m // 4][poff:poff + 64,
                                             (m % 4) * 128:(m % 4 + 1) * 128],
                            qt_tb[c][n][poff:poff + 64, :],
                            start=True, stop=True)
                    pt = ptp.tile([128, 1024], BF16, tag="ptp")
                    nc.scalar.activation(pt[:], sc[:], AF.Exp, scale=0.125)
                    pjidx = mp - 2 * n
                    if pjidx >= 0:
                        nc.vector.tensor_mul(pt[:], pt[:], masks[pjidx][:])
                    for half in range(2):
                        m = 2 * mp + half
                        nc.tensor.matmul(
                            ot[:],
                            vx_t[m // 4][:, h * 4 * (HD + 1) + (m % 4) * (HD + 1):
                                         h * 4 * (HD + 1) + (m % 4 + 1) * (HD + 1)],
                            pt[:, half * 512:(half + 1) * 512],
                            start=(m == 0), stop=(m == nkb - 1))
                # normalize: at[poff:poff+64] = ot[0:64] * (1/ot[64])
                den = rsp.tile([1, 512], F32, tag="den")
                nc.scalar.activation(den[:], ot[HD:HD + 1, :], AF.Copy)
                rsf = rsp.tile([1, 512], F32, tag="rsf")
                nc.vector.reciprocal_approx_fast(rsf[:], den[:])
                rbc = rsp.tile([64, 512], F32, tag="rbc")
                nc.gpsimd.partition_broadcast(rbc[:], rsf[:])
                nc.vector.tensor_mul(at_tb[c][n][poff:poff + 64, :],
                                     ot[0:HD, :], rbc[:])

        if debug:
            for c in range(2):
                for t in range(NTB):
                    nc.sync.dma_start(
                        qt_dbg.ap()[c, :, t * 512:(t + 1) * 512], qt_tb[c][t][:])
                    nc.sync.dma_start(
                        kt_dbg.ap()[c, :, t * 512:(t + 1) * 512], kt_tb[c][t][:])
                    nc.sync.dma_start(
                        at_dbg.ap()[c, :, t * 512:(t + 1) * 512], at_tb[c][t][:])
            for t in range(NTB):
                nc.sync.dma_start(
                    vx_dbg.ap()[:, t * HPC * 4 * (HD + 1):
                                (t + 1) * HPC * 4 * (HD + 1)], vx_t[t][:])

        # ---- phase 3: projection partial: a_part = aT^T @ Wproj ----
        for t in range(NTB128):
            n, off = t // 4, (t % 4) * 128
            for ob in range(2):
                pj = ph1.tile([128, 512], F32, tag="ph1")
                for cc in range(2):
                    nc.tensor.matmul(
                        pj[:],
                        at_tb[cc][n][:, off:off + 128],
                        wp_sb[:, cc * D + ob * 512: cc * D + (ob + 1) * 512],
                        start=(cc == 0), stop=(cc == 1))
                pst = stage.tile([128, 512], BF16, tag="stage")
                nc.vector.tensor_copy(pst[:], pj[:])
                nc.sync.dma_start(
                    a_d.ap()[ob, t * 128:(t + 1) * 128, :], pst[:])

        for p in (otp, scp, ph1, rsp, ptp, stage):
            p.release()
        for f in reversed(frees):
            f()

    nc.compile()
    return nc


_NC_CACHE = {}


def _get_program(with_bias=False):
    key = f"nc{int(with_bias)}"
    if key not in _NC_CACHE:
        _NC_CACHE[key] = _build_program(with_bias)
    return _NC_CACHE[key]


def kernel(x, c_attn_w, c_attn_b, c_proj_w, c_proj_b, q_a1, q_a2, v_a1, v_a2):
    x = np.asarray(x, np.float32)
    c_attn_w = np.asarray(c_attn_w, np.float32)
    c_attn_b = np.asarray(c_attn_b, np.float32)
    c_proj_w = np.asarray(c_proj_w, np.float32)
    c_proj_b = np.asarray(c_proj_b, np.float32)
    q_a1 = np.asarray(q_a1, np.float32)
    q_a2 = np.asarray(q_a2, np.float32)
    v_a1 = np.asarray(v_a1, np.float32)
    v_a2 = np.asarray(v_a2, np.float32)

    # Host prep: fold LoRA into the attention weights (exact).
    wq_eff = c_attn_w[:, :D] + LORA_SCALE * (q_a1 @ q_a2)
    wk_full = c_attn_w[:, D:2 * D]
    wv_eff = c_attn_w[:, 2 * D:] + LORA_SCALE * (v_a1 @ v_a2)
    qb_full, kb_full, vb_full = (c_attn_b[:D], c_attn_b[D:2 * D],
                                 c_attn_b[2 * D:])

    in_maps = []
    for core in range(N_CORES):
        b, g = divmod(core, 4)
        cs = slice(g * GC, (g + 1) * GC)
        in_maps.append({
            "xt": np.ascontiguousarray(
                x[b].T.astype(NPBF16).reshape(D, NTB, 512).transpose(1, 0, 2)),
            "wq": np.ascontiguousarray(wq_eff[:, cs].astype(NPBF16)),
            "wk": np.ascontiguousarray(wk_full[:, cs].astype(NPBF16)),
            "wv": np.ascontiguousarray(wv_eff[:, cs].astype(NPBF16)),
            "wp": np.ascontiguousarray(c_proj_w[cs, :].astype(NPBF16)),
            "qb": np.ascontiguousarray(
                qb_full[cs].reshape(2, 128).T.astype(np.float32)),
            "kb": np.ascontiguousarray(
                kb_full[cs].reshape(2, 128).T.astype(np.float32)),
        })

    with_bias = bool(np.any(c_attn_b[:2 * D]))
    nc = _get_program(with_bias)
    trace = bool(int(os.environ.get("BASSK_TRACE", "0")))
    res = run_bass_kernel_spmd(nc, in_maps, list(range(N_CORES)), trace=trace)
    if trace:
        _NC_CACHE["exec_time_ns"] = res.exec_time_ns
        _NC_CACHE["profile_json"] = res.profile_json
        _NC_CACHE["trace"] = res.instructions_and_trace

    # Host gather.
    a = np.zeros((B, S, D), np.float32)
    kh = np.empty((B, N_HEAD, S, HD), np.float32)
    vh = np.empty((B, N_HEAD, S, HD), np.float32)
    for core in range(N_CORES):
        b, g = divmod(core, 4)
        r = res.results[core]
        ap_bl = r["a_part"].astype(np.float32)
        a[b, :, 0:512] += ap_bl[0]
        a[b, :, 512:1024] += ap_bl[1]
        for hl in range(HPC):
            h = g * HPC + hl
            c, half = hl // 2, hl % 2
            kh[b, h] = r["k_out"][c][:, half * 64:(half + 1) * 64, :].transpose(
                0, 2, 1).reshape(S, HD).astype(np.float32)
            vh[b, h] = r["v_out"][:, hl * HD:(hl + 1) * HD].astype(np.float32)

    # Exact host-side bias folds: softmax rows sum to 1, so the v bias
    # passes through attention additively -> contributes vb @ c_proj_w.
    a += (c_proj_b + vb_full @ c_proj_w).astype(np.float32)
    vh += vb_full.reshape(N_HEAD, 1, HD)[None]
    present = np.stack((kh, vh)).astype(np.float32)
    return a, present


# revision 35
# speedup vs baseline: 1.1003x; 1.0915x over previous
"""Trainium2 Bass kernel for causal self-attention with LoRA (q,v adapters).

Full-input contract: kernel(**inputs) takes the unsharded numpy inputs and
returns (a, present) exactly like the reference.

Sharding: 8 cores = 2 batches x 4 head-groups (4 heads each).  Data parallel
on B, tensor parallel on heads: each core owns 256 columns of q/k/v and the
matching 256 rows of c_proj.  LoRA adapters are folded into c_attn_w on the
host (exact: q_eff = x @ (Wq + 0.5*q_a1@q_a2)).  The c_proj partial outputs
are summed on the host.

Device dataflow per core (all matmuls bf16 with fp32 PSUM accumulation):
  xT [1024,2048] -> QT,KT [256(cols),2048] transposed layout, V [2048,256]
  natural layout with a ones-column appended per head (softmax denominator
  comes out of the P@V matmul for free).
  Scores are computed transposed, ST[k,q] = KT_h^T slice . QT_h, so that
  P^T tiles feed the P@V matmul directly (contraction over k on partitions;
  no on-chip transposes anywhere in the kernel).
  exp(s/8) runs on ScalarE with the 1/sqrt(hd) scale folded in; causal
  masking multiplies the diagonal P^T tiles by 0/1 masks generated on-device.
  Normalization by the softmax denominator happens on the [65,512] P@V
  output via reciprocal + K=1 outer-product broadcast matmul.
  Finally projT: a_part = (aT)^T @ Wproj_rows accumulated in PSUM.

Bias handling (exact for arbitrary biases):
  q,k bias: added on device per-partition during the PSUM->SBUF copies.
  v bias: softmax rows sum to 1 so attention output shifts by exactly +bv;
  folded on host into `a` as bv @ c_proj_w, and added to present-v directly.
  proj bias: host-added.
"""

import os
import numpy as np
import ml_dtypes

import concourse.bass as bass
import concourse.tile as tile
from concourse.tile_rust import add_dep_helper
import concourse.mybir as mybir
from concourse import bacc
from concourse.bass_utils import run_bass_kernel_spmd

BF16 = mybir.dt.bfloat16
F32 = mybir.dt.float32
AF = mybir.ActivationFunctionType
NPBF16 = ml_dtypes.bfloat16

B, S, D = 2, 2048, 1024
N_HEAD = 16
HD = D // N_HEAD              # 64
LORA_SCALE = 16.0 / 32.0
N_CORES = 8
HPC = 4                       # heads per core
GC = HPC * HD                 # local columns per core = 256
KCH = D // 128                # contraction chunks = 8
NTB = S // 512                # 512-wide token blocks = 4
NTB128 = S // 128             # 128-wide token blocks = 16


def _build_program(with_bias=False):
    nc = bacc.Bacc("TRN2", target_bir_lowering=False, debug=False)

    # ---- per-core DRAM I/O ----
    xt_d = nc.dram_tensor("xt", [NTB, D, 512], BF16, kind="ExternalInput")
    wq_d = nc.dram_tensor("wq", [D, GC], BF16, kind="ExternalInput")
    wk_d = nc.dram_tensor("wk", [D, GC], BF16, kind="ExternalInput")
    wv_d = nc.dram_tensor("wv", [D, GC], BF16, kind="ExternalInput")
    wp_d = nc.dram_tensor("wp", [GC, D], BF16, kind="ExternalInput")
    qb_d = nc.dram_tensor("qb", [128, 2], F32, kind="ExternalInput")
    kb_d = nc.dram_tensor("kb", [128, 2], F32, kind="ExternalInput")

    a_d = nc.dram_tensor("a_part", [2, S, 512], BF16, kind="ExternalOutput")
    k_d = nc.dram_tensor("k_out", [2, NTB, 128, 512], BF16, kind="ExternalOutput")
    v_d = nc.dram_tensor("v_out", [S, GC], BF16, kind="ExternalOutput")
    debug = bool(int(os.environ.get("BASSK_DEBUG", "0")))
    if debug:
        qt_dbg = nc.dram_tensor("qt_dbg", [2, 128, S], BF16, kind="ExternalOutput")
        kt_dbg = nc.dram_tensor("kt_dbg", [2, 128, S], BF16, kind="ExternalOutput")
        vx_dbg = nc.dram_tensor("vx_dbg", [128, HPC * NTB128 * (HD + 1)], BF16,
                                kind="ExternalOutput")
        at_dbg = nc.dram_tensor("at_dbg", [2, 128, S], BF16, kind="ExternalOutput")

    with tile.TileContext(nc) as tc:
        # ---- persistent SBUF tiles, split per block so Tile's per-tile
        # dependency tracking lets phases pipeline into each other ----
        frees = []

        def single(shape, dtype, name):
            t, f = tc.tile(shape, dtype, name=name)
            frees.append(f)
            return t

        xt_b0 = single([128, KCH * 512], BF16, "xt_b0")
        xt_b1 = single([128, KCH * 512], BF16, "xt_b1")
        xt_rest = single([128, 2 * KCH * 512], BF16, "xt_rest")

        def xt_slice(t, k, off, width):
            base = k * 512 + off
            if t == 0:
                return xt_b0[:, base: base + width]
            if t == 1:
                return xt_b1[:, base: base + width]
            return xt_rest[:, (t - 2) * KCH * 512 + base:
                           (t - 2) * KCH * 512 + base + width]
        wq_sb = single([128, KCH * GC], BF16, "wq_sb")
        wk_sb = single([128, KCH * GC], BF16, "wk_sb")
        wv_sb = single([128, KCH * GC], BF16, "wv_sb")
        wp_sb = single([128, 2 * D], BF16, "wp_sb")
        qb_sb = single([128, 2], F32, "qb_sb")
        kb_sb = single([128, 2], F32, "kb_sb")
        qt_tb = [[single([128, 512], BF16, f"qt_tb{c}_{t}") for t in range(NTB)]
                 for c in range(2)]
        kt_tb = [[single([128, 512], BF16, f"kt_tb{c}_{t}") for t in range(NTB)]
                 for c in range(2)]
        # vx_t[t]: V rows for tokens [t*512,(t+1)*512), all 4 heads, with a
        # ones column per 128-chunk: layout [p, (h, c4, 65)]
        vx_t = [single([128, HPC * 4 * (HD + 1)], BF16, f"vx_t{t}")
                for t in range(NTB)]
        at_tb = [[single([128, 512], BF16, f"at_tb{c}_{n}") for n in range(NTB)]
                 for c in range(2)]
        masks = [single([128, 1024], BF16, f"mask{j}") for j in range(2)]

        # ---- transient pools ----
        stage = tc.alloc_tile_pool(name="stage", bufs=4)
        ptp = tc.alloc_tile_pool(name="ptp", bufs=8)
        rsp = tc.alloc_tile_pool(name="rsp", bufs=4)
        ph1 = tc.alloc_tile_pool(name="ph1", bufs=2, space="PSUM")
        scp = tc.alloc_tile_pool(name="scp", bufs=2, space="PSUM")
        otp = tc.alloc_tile_pool(name="otp", bufs=2, space="PSUM")

        # ---- constants ----
        for t in range(NTB):
            ones_slots = vx_t[t][:].rearrange(
                "p (h c w) -> p h c w", h=HPC, w=HD + 1)[:, :, :, HD]
            nc.vector.memset(ones_slots, 1.0)
        for pj in range(2):
            # halves hold mask_{2pj} and mask_{2pj+1};
            # mask_j[r, qq] = 1.0 iff qq >= j*128 + r else 0.0
            nc.gpsimd.memset(masks[pj][:], 1.0)
            for half in range(2):
                j = 2 * pj + half
                nc.gpsimd.affine_select(
                    out=masks[pj][:, half * 512:(half + 1) * 512],
                    in_=masks[pj][:, half * 512:(half + 1) * 512],
                    compare_op=mybir.AluOpType.is_ge,
                    fill=0.0, base=-128 * j,
                    pattern=[[1, 512]], channel_multiplier=-1,
                )

        # ---- input DMAs: xt block 0 gets its own transfer so it lands
        # first; blocks 1-3 stream as one big transfer behind it ----
        for w_sb, w_d in ((wq_sb, wq_d), (wk_sb, wk_d)):
            nc.sync.dma_start(
                w_sb[:].rearrange("p (k n) -> p k n", n=GC),
                w_d.ap().rearrange("(k p) n -> p k n", p=128))
        d0 = nc.sync.dma_start(
            xt_b0[:].rearrange("p (k w) -> p k w", w=512),
            xt_d.ap()[0].rearrange("(k p) w -> p k w", p=128))
        d1 = nc.sync.dma_start(
            xt_b1[:].rearrange("p (k w) -> p k w", w=512),
            xt_d.ap()[1].rearrange("(k p) w -> p k w", p=128))
        dr = nc.sync.dma_start(
            xt_rest[:].rearrange("p (t k w) -> p t k w", t=2, w=512),
            xt_d.ap()[2:].rearrange("t (k p) w -> p t k w", p=128))
        # stagger everything behind xt block 0 so its completion (which
        # gates the first matmuls) gets the full HBM bandwidth
        add_dep_helper(d1.ins, d0.ins, sync=True, reason="xt b1 after b0")
        add_dep_helper(dr.ins, d1.ins, sync=True, reason="xt rest after b1")
        dv = nc.scalar.dma_start(
            wv_sb[:].rearrange("p (k n) -> p k n", n=GC),
            wv_d.ap().rearrange("(k p) n -> p k n", p=128))
        dp = nc.gpsimd.dma_start(
            wp_sb[:].rearrange("p (k n) -> p k n", n=D),
            wp_d.ap().rearrange("(k p) n -> p k n", p=128))
        add_dep_helper(dv.ins, d0.ins, sync=True, reason="wv after xt b0")
        add_dep_helper(dp.ins, d0.ins, sync=True, reason="wp after xt b0")
        if with_bias:
            nc.sync.dma_start(qb_sb[:], qb_d.ap())
            nc.sync.dma_start(kb_sb[:], kb_d.ap())

        # ---- phase 1: QT, KT, V interleaved per 512-token block ----
        for t in range(NTB):
            for c in range(2):
                psq = ph1.tile([128, 512], F32, tag="ph1")
                for k in range(KCH):
                    nc.tensor.matmul(
                        psq[:],
                        wq_sb[:, k * GC + c * 128: k * GC + (c + 1) * 128],
                        xt_slice(t, k, 0, 512),
                        start=(k == 0), stop=(k == KCH - 1))
                if with_bias:
                    nc.scalar.activation(qt_tb[c][t][:], psq[:], AF.Identity,
                                         bias=qb_sb[:, c: c + 1])
                else:
                    nc.vector.tensor_copy(qt_tb[c][t][:], psq[:])

                psk = ph1.tile([128, 512], F32, tag="ph1")
                for k in range(KCH):
                    nc.tensor.matmul(
                        psk[:],
                        wk_sb[:, k * GC + c * 128: k * GC + (c + 1) * 128],
                        xt_slice(t, k, 0, 512),
                        start=(k == 0), stop=(k == KCH - 1))
                if with_bias:
                    nc.scalar.activation(kt_tb[c][t][:], psk[:], AF.Identity,
                                         bias=kb_sb[:, c: c + 1])
                else:
                    nc.vector.tensor_copy(kt_tb[c][t][:], psk[:])
                nc.sync.dma_start(k_d.ap()[c, t], kt_tb[c][t][:])

            for q4 in range(4):
                tt = 4 * t + q4
                psv = ph1.tile([128, GC], F32, tag="ph1")
                for k in range(KCH):
                    nc.tensor.matmul(
                        psv[:],
                        xt_slice(t, k, q4 * 128, 128),
                        wv_sb[:, k * GC:(k + 1) * GC],
                        start=(k == 0), stop=(k == KCH - 1))
                vx_dst = vx_t[t][:].rearrange(
                    "p (h c w) -> p h c w", h=HPC, w=HD + 1)[:, :, q4, 0:HD]
                nc.vector.tensor_copy(
                    vx_dst, psv[:].rearrange("p (h w) -> p h w", w=HD))
                vst = stage.tile([128, GC], BF16, tag="stage")
                nc.vector.tensor_copy(vst[:], psv[:])
                nc.sync.dma_start(v_d.ap()[tt * 128:(tt + 1) * 128, :], vst[:])

        # ---- phase 2: causal attention, n-outer h-inner so each head's
        # normalize chain overlaps the other heads' matmuls ----
        for n in range(NTB):
            for h in range(HPC):
                c, poff = h // 2, (h % 2) * 64
                nkb = 4 * n + 4
                ot = otp.tile([HD + 1, 512], F32, tag="otp")
                for mp in range(nkb // 2):
                    boundary = (mp - 2 * n) >= 0
                    sc = scp.tile([128, 1024], F32, tag="scp")
                    pt = ptp.tile([128, 1024], BF16, tag="ptp")
                    if not boundary:
                        for half in range(2):
                            m = 2 * mp + half
                            nc.tensor.matmul(
                                sc[:, half * 512:(half + 1) * 512],
                                kt_tb[c][m // 4][poff:poff + 64,
                                                 (m % 4) * 128:(m % 4 + 1) * 128],
                                qt_tb[c][n][poff:poff + 64, :],
                                start=True, stop=True)
                        nc.scalar.activation(pt[:], sc[:], AF.Exp, scale=0.125)
                    else:
                        # diagonal pair: leading j*128 q-columns of half (with
                        # j = m - 4n) are fully masked -- skip their scores
                        # and exp, zero them in pt, and mask only the 128-wide
                        # triangular stripe
                        for half in range(2):
                            m = 2 * mp + half
                            j = m - 4 * n
                            z = j * 128
                            nc.tensor.matmul(
                                sc[:, half * 512 + z:(half + 1) * 512],
                                kt_tb[c][m // 4][poff:poff + 64,
                                                 (m % 4) * 128:(m % 4 + 1) * 128],
                                qt_tb[c][n][poff:poff + 64, z:512],
                                start=True, stop=True)
                            if z:
                                nc.vector.memset(
                                    pt[:, half * 512: half * 512 + z], 0.0)
                            nc.scalar.activation(
                                pt[:, half * 512 + z:(half + 1) * 512],
                                sc[:, half * 512 + z:(half + 1) * 512],
                                AF.Exp, scale=0.125)
                            nc.vector.tensor_mul(
                                pt[:, half * 512 + z: half * 512 + z + 128],
                                pt[:, half * 512 + z: half * 512 + z + 128],
                                masks[0][:, 0:128])
                    for half in range(2):
                        m = 2 * mp + half
                        nc.tensor.matmul(
                            ot[:],
                            vx_t[m // 4][:, h * 4 * (HD + 1) + (m % 4) * (HD + 1):
                                         h * 4 * (HD + 1) + (m % 4 + 1) * (HD + 1)],
                            pt[:, half * 512:(half + 1) * 512],
                            start=(m == 0), stop=(m == nkb - 1))
                # normalize: at[poff:poff+64] = ot[0:64] * (1/ot[64])
                den = rsp.tile([1, 512], F32, tag="den")
                nc.vector.tensor_copy(den[:], ot[HD:HD + 1, :])
                rsf = rsp.tile([1, 512], F32, tag="rsf")
                nc.vector.reciprocal_approx_fast(rsf[:], den[:])
                rbc = rsp.tile([64, 512], F32, tag="rbc")
                nc.gpsimd.partition_broadcast(rbc[:], rsf[:])
                nc.vector.tensor_mul(at_tb[c][n][poff:poff + 64, :],
                                     ot[0:HD, :], rbc[:])

        if debug:
            for c in range(2):
                for t in range(NTB):
                    nc.sync.dma_start(
                        qt_dbg.ap()[c, :, t * 512:(t + 1) * 512], qt_tb[c][t][:])
                    nc.sync.dma_start(
                        kt_dbg.ap()[c, :, t * 512:(t + 1) * 512], kt_tb[c][t][:])
                    nc.sync.dma_start(
                        at_dbg.ap()[c, :, t * 512:(t + 1) * 512], at_tb[c][t][:])
            for t in range(NTB):
                nc.sync.dma_start(
                    vx_dbg.ap()[:, t * HPC * 4 * (HD + 1):
                                (t + 1) * HPC * 4 * (HD + 1)], vx_t[t][:])

        # ---- phase 3: projection partial: a_part = aT^T @ Wproj ----
        for t in range(NTB128):
            n, off = t // 4, (t % 4) * 128
            for ob in range(2):
                pj = ph1.tile([128, 512], F32, tag="ph1")
                for cc in range(2):
                    nc.tensor.matmul(
                        pj[:],
                        at_tb[cc][n][:, off:off + 128],
                        wp_sb[:, cc * D + ob * 512: cc * D + (ob + 1) * 512],
                        start=(cc == 0), stop=(cc == 1))
                pst = stage.tile([128, 512], BF16, tag="stage")
                nc.vector.tensor_copy(pst[:], pj[:])
                nc.sync.dma_start(
                    a_d.ap()[ob, t * 128:(t + 1) * 128, :], pst[:])

        for p in (otp, scp, ph1, rsp, ptp, stage):
            p.release()
        for f in reversed(frees):
            f()

    nc.compile()
    return nc


_NC_CACHE = {}


def _get_program(with_bias=False):
    key = f"nc{int(with_bias)}"
    if key not in _NC_CACHE:
        _NC_CACHE[key] = _build_program(with_bias)
    return _NC_CACHE[key]


def kernel(x, c_attn_w, c_attn_b, c_proj_w, c_proj_b, q_a1, q_a2, v_a1, v_a2):
    x = np.asarray(x, np.float32)
    c_attn_w = np.asarray(c_attn_w, np.float32)
    c_attn_b = np.asarray(c_attn_b, np.float32)
    c_proj_w = np.asarray(c_proj_w, np.float32)
    c_proj_b = np.asarray(c_proj_b, np.float32)
    q_a1 = np.asarray(q_a1, np.float32)
    q_a2 = np.asarray(q_a2, np.float32)
    v_a1 = np.asarray(v_a1, np.float32)
    v_a2 = np.asarray(v_a2, np.float32)

    # Host prep: fold LoRA into the attention weights (exact).
    wq_eff = c_attn_w[:, :D] + LORA_SCALE * (q_a1 @ q_a2)
    wk_full = c_attn_w[:, D:2 * D]
    wv_eff = c_attn_w[:, 2 * D:] + LORA_SCALE * (v_a1 @ v_a2)
    qb_full, kb_full, vb_full = (c_attn_b[:D], c_attn_b[D:2 * D],
                                 c_attn_b[2 * D:])

    in_maps = []
    for core in range(N_CORES):
        b, g = divmod(core, 4)
        cs = slice(g * GC, (g + 1) * GC)
        in_maps.append({
            "xt": np.ascontiguousarray(
                x[b].T.astype(NPBF16).reshape(D, NTB, 512).transpose(1, 0, 2)),
            "wq": np.ascontiguousarray(wq_eff[:, cs].astype(NPBF16)),
            "wk": np.ascontiguousarray(wk_full[:, cs].astype(NPBF16)),
            "wv": np.ascontiguousarray(wv_eff[:, cs].astype(NPBF16)),
            "wp": np.ascontiguousarray(c_proj_w[cs, :].astype(NPBF16)),
            "qb": np.ascontiguousarray(
                qb_full[cs].reshape(2, 128).T.astype(np.float32)),
            "kb": np.ascontiguousarray(
                kb_full[cs].reshape(2, 128).T.astype(np.float32)),
        })

    with_bias = bool(np.any(c_attn_b[:2 * D]))
    nc = _get_program(with_bias)
    trace = bool(int(os.environ.get("BASSK_TRACE", "0")))
    res = run_bass_kernel_spmd(nc, in_maps, list(range(N_CORES)), trace=trace)
    if trace:
        _NC_CACHE["exec_time_ns"] = res.exec_time_ns
        _NC_CACHE["profile_json"] = res.profile_json
        _NC_CACHE["trace"] = res.instructions_and_trace

    # Host gather.
    a = np.zeros((B, S, D), np.float32)
    kh = np.empty((B, N_HEAD, S, HD), np.float32)
    vh = np.empty((B, N_HEAD, S, HD), np.float32)
    for core in range(N_CORES):
        b, g = divmod(core, 4)
        r = res.results[core]
        ap_bl = r["a_part"].astype(np.float32)
        a[b, :, 0:512] += ap_bl[0]
        a[b, :, 512:1024] += ap_bl[1]
        for hl in range(HPC):
            h = g * HPC + hl
            c, half = hl // 2, hl % 2
            kh[b, h] = r["k_out"][c][:, half * 64:(half + 1) * 64, :].transpose(
                0, 2, 1).reshape(S, HD).astype(np.float32)
            vh[b, h] = r["v_out"][:, hl * HD:(hl + 1) * HD].astype(np.float32)

    # Exact host-side bias folds: softmax rows sum to 1, so the v bias
    # passes through attention additively -> contributes vb @ c_proj_w.
    a += (c_proj_b + vb_full @ c_proj_w).astype(np.float32)
    vh += vb_full.reshape(N_HEAD, 1, HD)[None]
    present = np.stack((kh, vh)).astype(np.float32)
    return a, present
